# revision 3
# baseline (speedup 1.0000x reference)
"""LSTM cell (batch 8192, input 512, hidden 512) on 8 Trainium2 NeuronCores.

Data-parallel over the batch dim: each core handles 1024 rows; weights are
replicated. Everything is computed in [hidden, batch] layout with the
contraction dim (fan_in = 1024) on SBUF partitions:

  gate.T[n, b] = sum_k W.T[k, n] * combined.T[k, b]     (matmul: lhsT.T @ rhs)

Precision plan (measured rel-err 1.5e-2 vs the 2e-2 budget): the i-gate
matmul runs in fp8-e4m3 with MatmulPerfMode.DoubleRow (K=256 per
instruction, 2x MAC rate -- one DR matmul costs the same ~216ns as one
K=128 bf16 matmul), with weights pre-scaled by 128 (descaled via the ACT
scale operand). The f/c/o gates stay bf16: their error amplification
(f multiplies c_prev, c~ passes tanh' = 1, o hits h directly) makes fp8
too lossy for them. Gate activations + the elementwise tail are f32;
c_prev is shipped bf16.

Per-core PE work: 8 groups x (4 DR + 24 bf16 matmuls) x 512 cols
= 114688 cycles ~ 47.8us at 2.4GHz, vs 54.6us for all-bf16.

DMA ramp: the fp8 activations (1MB) stream first so the i-gate DR matmuls
start after ~0.3MB lands; bf16 activations (2MB) follow and are consumed
k-major by the h=0 c/f/o matmuls; weights ride separate rings (scalar:
fp8 + h0 strips, vector: h1-3 strips) in (k,h)-granular pieces so h=0
needs only 0.75MB of bf16 weights in flight.
"""

import numpy as np

import concourse.bacc as bacc
import concourse.bass as bass
import concourse.mybir as mybir
from concourse import tile
from concourse.bass_utils import run_bass_kernel_spmd

N_CORES = 8
BATCH = 8192
B = BATCH // N_CORES  # 1024 batch rows per core
K = 1024              # fan_in = input_dim + hidden_dim
H = 512               # hidden dim
KT = K // 128         # 8 bf16 contraction tiles
KP = K // 256         # 4 fp8 DoubleRow contraction tiles
HT = H // 128         # 4 hidden chunks per gate
BT = B // 512         # 2 batch halves (PSUM free-dim limit is 512 f32)
SW = 128.0            # fp8 weight pre-scale (descaled in ACT)

E4 = mybir.dt.float8e4
BF = mybir.dt.bfloat16
F32 = mybir.dt.float32
DR = mybir.MatmulPerfMode.DoubleRow

_SIG = mybir.ActivationFunctionType.Sigmoid
_TANH = mybir.ActivationFunctionType.Tanh
# gate order in this kernel: 0=i (fp8), 1=c, 2=f, 3=o; bias2d col = g*HT + h
_GATE_FN = [_SIG, _TANH, _SIG, _SIG]


def _build():
    nc = bacc.Bacc(
        "TRN2",
        target_bir_lowering=False,
        debug=False,
        num_devices=N_CORES,
    )

    # a8 rows kp*128+p, cols j*B + b  (j = which 128-half of the 256 k-block)
    a8 = nc.dram_tensor("a8", [KP * 128, 2 * B], E4, kind="ExternalInput")
    # a16 rows k*128+p, cols b
    a16 = nc.dram_tensor("a16", [K, B], BF, kind="ExternalInput")
    # w8i rows kp*128+p, cols h*256 + j*128 + m
    w8i = nc.dram_tensor("w8i", [KP * 128, HT * 256], E4, kind="ExternalInput")
    # w16 rows k*128+p, cols h*384 + gi*128 + m  (gi: 0=c, 1=f, 2=o)
    w16 = nc.dram_tensor("w16", [K, HT * 384], BF, kind="ExternalInput")
    bias2d = nc.dram_tensor("bias2d", [128, 4 * HT], F32, kind="ExternalInput")
    cp16 = nc.dram_tensor("cp16", [H, B], BF, kind="ExternalInput")
    h_nextT = nc.dram_tensor("h_nextT", [H, B], F32, kind="ExternalOutput")
    c_nextT = nc.dram_tensor("c_nextT", [H, B], F32, kind="ExternalOutput")

    with tile.TileContext(nc) as tc:
        with (
            tc.tile_pool(name="acts", bufs=1) as apool,
            tc.tile_pool(name="wts", bufs=1) as wpool,
            tc.tile_pool(name="cprev", bufs=1) as cpool,
            tc.tile_pool(name="gates", bufs=3) as gpool,
            tc.tile_pool(name="ew", bufs=3) as epool,
            tc.tile_pool(name="psum", bufs=1, space="PSUM") as pspool,
        ):
            # --- input streams -------------------------------------------
            # sync ring: fp8 activations first (i-gate starts on them), then
            # bf16 activations k-major for the h=0 c/f/o ramp.
            a8_t = []
            for kp in range(KP):
                t = apool.tile([128, 2, B], E4, tag=f"a8_{kp}", name=f"a8_{kp}")
                nc.sync.dma_start(t[:], a8[kp * 128:(kp + 1) * 128, :])
                a8_t.append(t)
            a16_t = []
            for k in range(KT):
                t = apool.tile([128, B], BF, tag=f"a16_{k}", name=f"a16_{k}")
                nc.sync.dma_start(t[:], a16[k * 128:(k + 1) * 128, :])
                a16_t.append(t)

            # scalar ring: bias, fp8 weights, then the h=0 bf16 strips.
            bias_t = wpool.tile([128, 4 * HT], F32, tag="bias", name="bias")
            nc.scalar.dma_start(bias_t[:], bias2d[:])
            w8i_t = []
            for kp in range(KP):
                t = wpool.tile([128, HT, 2, 128], E4, tag=f"w8_{kp}", name=f"w8_{kp}")
                nc.scalar.dma_start(t[:], w8i[kp * 128:(kp + 1) * 128, :])
                w8i_t.append(t)

            w16_t = [[None] * HT for _ in range(KT)]

            def _load_w16(k, h, eng):
                t = wpool.tile([128, 3, 128], BF, tag=f"w16_{k}_{h}", name=f"w16_{k}_{h}")
                eng.dma_start(t[:], w16[k * 128:(k + 1) * 128, h * 384:(h + 1) * 384])
                w16_t[k][h] = t

            for k in range(KT):
                _load_w16(k, 0, nc.scalar)
            # sync ring, queued behind the activation stream: h>=1 strips
            # transfer after a16 lands (~10us) and are first consumed at
            # ~13us, so they never block the ramp.
            for h in range(1, HT):
                for k in range(KT):
                    _load_w16(k, h, nc.sync)

            # gpsimd ring: c_prev (first needed by the h=0 tail at ~6us).
            cp_t = []
            for h in range(HT):
                t = cpool.tile([128, B], BF, tag=f"cp_{h}", name=f"cp_{h}")
                nc.gpsimd.dma_start(t[:], cp16[h * 128:(h + 1) * 128, :])
                cp_t.append(t)

            # --- compute -------------------------------------------------
            def _mk_psum(g, par):
                return pspool.tile([128, 512], F32, tag=f"ps{g}_{par}",
                                   name=f"ps{g}_{par}")

            def _mm_i(ps, kp, h, b2):
                nc.tensor.matmul(
                    ps[:],
                    w8i_t[kp][:, h, :, :],
                    a8_t[kp][:, :, b2 * 512:(b2 + 1) * 512],
                    start=(kp == 0), stop=(kp == KP - 1),
                    perf_mode=DR,
                )

            def _mm_g(ps, gi, k, h, b2):
                nc.tensor.matmul(
                    ps[:],
                    w16_t[k][h][:, gi, :],
                    a16_t[k][:, b2 * 512:(b2 + 1) * 512],
                    start=(k == 0), stop=(k == KT - 1),
                )

            def _tail(h, b2, psum, chunks=1, dma_eng=None):
                """ACT + LSTM cell tail for one (h, b2) group.

                psum = [i, c, f, o] banks. chunks>1 splits the free dim so
                the final group's serial ACT->DVE chain drains in pieces.
                """
                dma_eng = dma_eng or nc.gpsimd
                hs = slice(h * 128, (h + 1) * 128)
                w = 512 // chunks

                def _act(g, c, scale):
                    t = gpool.tile([128, w], F32, tag=f"g{g}", name=f"g{g}_{h}_{b2}_{c}")
                    nc.scalar.activation(
                        t[:], psum[g][:, c * w:(c + 1) * w], _GATE_FN[g],
                        bias=bias_t[:, g * HT + h:g * HT + h + 1],
                        scale=scale,
                    )
                    return t

                for c in range(chunks):
                    cs = slice(b2 * 512 + c * w, b2 * 512 + (c + 1) * w)
                    gi = _act(0, c, 1.0 / SW)
                    gc = _act(1, c, 1.0)
                    gf = _act(2, c, 1.0)

                    t1 = epool.tile([128, w], F32, tag="t1", name=f"t1_{h}_{b2}_{c}")
                    nc.vector.tensor_mul(t1[:], gi[:], gc[:])       # i * c~
                    t2 = epool.tile([128, w], F32, tag="t2", name=f"t2_{h}_{b2}_{c}")
                    nc.vector.tensor_mul(t2[:], gf[:], cp_t[h][:, cs])
                    cn = epool.tile([128, w], F32, tag="cn", name=f"cn_{h}_{b2}_{c}")
                    nc.vector.tensor_add(cn[:], t1[:], t2[:])
                    dma_eng.dma_start(c_nextT[hs, cs], cn[:])

                    th = epool.tile([128, w], F32, tag="th", name=f"th_{h}_{b2}_{c}")
                    nc.scalar.activation(th[:], cn[:], _TANH)

                    go = _act(3, c, 1.0)
                    hn = epool.tile([128, w], F32, tag="hn", name=f"hn_{h}_{b2}_{c}")
                    nc.vector.tensor_mul(hn[:], go[:], th[:])
                    dma_eng.dma_start(h_nextT[hs, cs], hn[:])

            # h=0 rides the DMA ramp: all 8 banks (4 gates x 2 halves), the
            # i-gate kp-major on the fp8 stream, then c/f/o k-major on the
            # bf16 stream so each a16 tile is consumed 6 matmuls at a time
            # right as it lands.
            psum0 = {b2: [_mk_psum(g, b2) for g in range(4)] for b2 in range(BT)}
            for kp in range(KP):
                for b2 in range(BT):
                    _mm_i(psum0[b2][0], kp, 0, b2)
            for k in range(KT):
                for g in range(1, 4):
                    for b2 in range(BT):
                        _mm_g(psum0[b2][g], g - 1, k, 0, b2)
            for b2 in range(BT):
                _tail(0, b2, psum0[b2])

            # h>=1: inputs resident; per-(h,b2) 4-bank groups with parity
            # alternating so each set's ACT drain overlaps the other's
            # matmuls. Gate order i, c, f, o: the tail's dependency chain
            # (t1 = i*c~, t2 = f*cp, h = o*tanh(c)) starts draining while
            # the o-gate matmuls are still on the PE.
            for h in range(1, HT):
                for b2 in range(BT):
                    par = (h * BT + b2) % 2
                    psum = [_mk_psum(g, par) for g in range(4)]
                    for kp in range(KP):
                        _mm_i(psum[0], kp, h, b2)
                    for g in range(1, 4):
                        for k in range(KT):
                            _mm_g(psum[g], g - 1, k, h, b2)
                    last = (h == HT - 1 and b2 == BT - 1)
                    _tail(h, b2, psum,
                          chunks=4 if last else 1,
                          dma_eng=nc.sync if last else None)

    nc.compile()
    return nc


_NC_CACHE = None
_LAST_IN_MAPS = None


def kernel(x, h_prev, c_prev, W_i, b_i, W_f, b_f, W_c, b_c, W_o, b_o):
    global _NC_CACHE, _LAST_IN_MAPS
    if _NC_CACHE is None:
        _NC_CACHE = _build()
    nc = _NC_CACHE

    np_e4 = mybir.dt.np(E4)
    np_bf = mybir.dt.np(BF)

    combT = np.concatenate([x, h_prev], axis=1).T          # [K, BATCH] f32
    a8_full = combT.astype(np_e4)
    a16_full = combT.astype(np_bf)

    # a8 per-core layout [kp*128+p, j*B+b] <- combT[kp*256+j*128+p, b]
    # built per core below from the column slice.

    # w8i[kp*128+p, h*256+j*128+m] = (W_i*SW)[h*128+m, kp*256+j*128+p]
    w8i = np.ascontiguousarray(
        (W_i * SW).astype(np_e4)
        .reshape(HT, 128, KP, 2, 128)      # [h, m, kp, j, p]
        .transpose(2, 4, 0, 3, 1)          # [kp, p, h, j, m]
        .reshape(KP * 128, HT * 256)
    )
    # w16[k*128+p, h*384+gi*128+m] = W_g[h*128+m, k*128+p], gi order (c, f, o)
    w16 = np.ascontiguousarray(
        np.stack([W_c, W_f, W_o])
        .astype(np_bf)
        .reshape(3, HT, 128, KT, 128)      # [gi, h, m, k, p]
        .transpose(3, 4, 1, 0, 2)          # [k, p, h, gi, m]
        .reshape(K, HT * 384)
    )
    # bias2d[m, g*HT+h] = b_g[h*128+m], gate order (i, c, f, o)
    bias2d = np.ascontiguousarray(
        np.stack([b_i, b_c, b_f, b_o])
        .reshape(4, HT, 128)
        .transpose(2, 0, 1)
        .reshape(128, 4 * HT)
    ).astype(np.float32)
    cp_full = c_prev.T.astype(np_bf)                       # [H, BATCH]

    in_maps = []
    for j in range(N_CORES):
        cols = slice(j * B, (j + 1) * B)
        a8_core = np.ascontiguousarray(
            a8_full[:, cols].reshape(KP, 2, 128, B)       # [kp, j2, p, b]
            .transpose(0, 2, 1, 3)                        # [kp, p, j2, b]
            .reshape(KP * 128, 2 * B)
        )
        in_maps.append({
            "a8": a8_core,
            "a16": np.ascontiguousarray(a16_full[:, cols]),
            "w8i": w8i,
            "w16": w16,
            "bias2d": bias2d,
            "cp16": np.ascontiguousarray(cp_full[:, cols]),
        })

    _LAST_IN_MAPS = in_maps
    try:
        res = run_bass_kernel_spmd(nc, in_maps, core_ids=list(range(N_CORES)))
    except Exception:
        # transient NRT_EXEC_UNIT_UNRECOVERABLE has been observed once on an
        # otherwise-correct NEFF; one retry is cheap insurance.
        res = run_bass_kernel_spmd(nc, in_maps, core_ids=list(range(N_CORES)))

    h_next = np.concatenate([r["h_nextT"].T for r in res.results], axis=0)
    c_next = np.concatenate([r["c_nextT"].T for r in res.results], axis=0)
    return (h_next.astype(np.float32), c_next.astype(np.float32))


# revision 7
# speedup vs baseline: 1.0067x; 1.0067x over previous
"""LSTM cell (batch 8192, input 512, hidden 512) on 8 Trainium2 NeuronCores.

Data-parallel over the batch dim: each core handles 1024 rows; weights are
replicated. Everything is computed in [hidden, batch] layout with the
contraction dim (fan_in = 1024) on SBUF partitions:

  gate.T[n, b] = sum_k W.T[k, n] * combined.T[k, b]     (matmul: lhsT.T @ rhs)

Precision plan (measured rel-err 1.55e-2 vs the 2e-2 budget): the i-gate
matmul runs in fp8-e4m3 with MatmulPerfMode.DoubleRow (K=256 per
instruction at the same ~216ns as a K=128 bf16 matmul = 2x MACs), weights
pre-scaled by 128 and descaled via the ACT scale operand. The f/c/o gates
stay bf16: their error amplification (f multiplies c_prev, c~ passes
tanh' = 1, o hits h directly) makes fp8 too lossy for them. Gates are f32;
cn/th/hn and both outputs are bf16; c_prev ships bf16.

Schedule facts this version is built around (measured on HW):
- PE dtype switches (bf16 <-> fp8) cost ~400ns of PE stall, so ALL fp8
  work runs as one up-front phase (i-gates of all 8 groups, parked in
  SBUF), then one pure-bf16 phase. 1 switch instead of 16.
- Engine-issued DMAs serialize through one global FIFO (~275 GB/s) in
  issue order, so issue order == need order: w8i/a8 pairs (phase 1)
  first, then (a16[k], w16[k][h0]) pairs k-major feeding the h=0 bf16
  ramp, then per-h weight strips + c_prev, outputs on their own ring.
- Per-core PE floor: 32 DR + 192 bf16 matmuls x 512 cols ~ 48.5us.
"""

import numpy as np

import concourse.bacc as bacc
import concourse.bass as bass
import concourse.mybir as mybir
from concourse import tile
from concourse.bass_utils import run_bass_kernel_spmd

N_CORES = 8
BATCH = 8192
B = BATCH // N_CORES  # 1024 batch rows per core
K = 1024              # fan_in = input_dim + hidden_dim
H = 512               # hidden dim
KT = K // 128         # 8 bf16 contraction tiles
KP = K // 256         # 4 fp8 DoubleRow contraction tiles
HT = H // 128         # 4 hidden chunks per gate
BT = B // 512         # 2 batch halves (PSUM free-dim limit is 512 f32)
SW = 128.0            # fp8 weight pre-scale (descaled in ACT)

E4 = mybir.dt.float8e4
BF = mybir.dt.bfloat16
F32 = mybir.dt.float32
DR = mybir.MatmulPerfMode.DoubleRow

_SIG = mybir.ActivationFunctionType.Sigmoid
_TANH = mybir.ActivationFunctionType.Tanh


def _build():
    nc = bacc.Bacc(
        "TRN2",
        target_bir_lowering=False,
        debug=False,
        num_devices=N_CORES,
    )

    # a8 rows kp*128+p, cols j*B + b  (j = which 128-half of the 256 k-block)
    a8 = nc.dram_tensor("a8", [KP * 128, 2 * B], E4, kind="ExternalInput")
    # a16 rows k*128+p, cols b
    a16 = nc.dram_tensor("a16", [K, B], BF, kind="ExternalInput")
    # w8i rows kp*128+p, cols h*256 + j*128 + m
    w8i = nc.dram_tensor("w8i", [KP * 128, HT * 256], E4, kind="ExternalInput")
    # w16 rows p, cols h*3072 + k*384 + gi*128 + m  (gi: 0=c, 1=f, 2=o)
    w16 = nc.dram_tensor("w16", [128, HT * KT * 384], BF, kind="ExternalInput")
    # bias2d col = g*HT + h, gate order (i, c, f, o)
    bias2d = nc.dram_tensor("bias2d", [128, 4 * HT], F32, kind="ExternalInput")
    cp16 = nc.dram_tensor("cp16", [H, B], BF, kind="ExternalInput")
    h_nextT = nc.dram_tensor("h_nextT", [H, B], BF, kind="ExternalOutput")
    c_nextT = nc.dram_tensor("c_nextT", [H, B], BF, kind="ExternalOutput")

    with tile.TileContext(nc) as tc:
        with (
            tc.tile_pool(name="acts", bufs=1) as apool,
            tc.tile_pool(name="wts", bufs=1) as wpool,
            tc.tile_pool(name="igates", bufs=1) as ipool,
            tc.tile_pool(name="gates", bufs=3) as gpool,
            tc.tile_pool(name="ew", bufs=3) as epool,
            tc.tile_pool(name="psum", bufs=1, space="PSUM") as pspool,
        ):
            # --- input DMA, in need order ---------------------------------
            # scalar ring: bias + fp8 weights (phase-1 critical path; the
            # scalar engine is otherwise idle until the first i-gate ACT).
            bias_t = wpool.tile([128, 4 * HT], F32, tag="bias", name="bias")
            nc.scalar.dma_start(bias_t[:], bias2d[:])
            w8i_t = []
            for kp in range(KP):
                t = wpool.tile([128, HT, 2, 128], E4, tag=f"w8_{kp}", name=f"w8_{kp}")
                nc.scalar.dma_start(t[:], w8i[kp * 128:(kp + 1) * 128, :])
                w8i_t.append(t)

            # sync ring: fp8 activations, then (a16[k], w16[k][h0]) pairs
            # k-major -- the h=0 bf16 phase consumes exactly in this order --
            # then the batched h>=1 weight strips.
            a8_t = []
            for kp in range(KP):
                t = apool.tile([128, 2, B], E4, tag=f"a8_{kp}", name=f"a8_{kp}")
                nc.sync.dma_start(t[:], a8[kp * 128:(kp + 1) * 128, :])
                a8_t.append(t)
            a16_t = [None] * KT
            w16h0_t = [None] * KT
            for k in range(KT):
                t = apool.tile([128, B], BF, tag=f"a16_{k}", name=f"a16_{k}")
                nc.sync.dma_start(t[:], a16[k * 128:(k + 1) * 128, :])
                a16_t[k] = t
                wt = wpool.tile([128, 3, 128], BF, tag=f"w16h0_{k}", name=f"w16h0_{k}")
                nc.sync.dma_start(wt[:], w16[:, k * 384:(k + 1) * 384])
                w16h0_t[k] = wt
            # h>=1 strips: one batched DMA per h ([k*128+p] rows gathered
            # into partition p with k on a free axis).
            w16h_t = [None] * HT
            for h in range(1, HT):
                wt = wpool.tile([128, KT, 3, 128], BF, tag=f"w16h_{h}", name=f"w16h_{h}")
                nc.sync.dma_start(wt[:], w16[:, h * 3072:(h + 1) * 3072])
                w16h_t[h] = wt

            def _w16(k, h, gi):
                if h == 0:
                    return w16h0_t[k][:, gi, :]
                return w16h_t[h][:, k, gi, :]

            # --- phase 1: all i-gate fp8 DoubleRow matmuls ---------------
            # Results parked as f32 SBUF tiles; ~7us of PE, one dtype
            # switch when the bf16 phase starts.
            i_t = [[None] * BT for _ in range(HT)]

            def _ps_i(par):
                return pspool.tile([128, 512], F32, tag=f"psI{par}", name=f"psI{par}")

            def _mm_i(ps, kp, h, b2):
                nc.tensor.matmul(
                    ps[:],
                    w8i_t[kp][:, h, :, :],
                    a8_t[kp][:, :, b2 * 512:(b2 + 1) * 512],
                    start=(kp == 0), stop=(kp == KP - 1),
                    perf_mode=DR,
                )

            def _act_i(ps, h, b2):
                t = ipool.tile([128, 512], F32, tag=f"i_{h}_{b2}", name=f"i_{h}_{b2}")
                nc.scalar.activation(
                    t[:], ps[:], _SIG,
                    bias=bias_t[:, h:h + 1],  # gate 0 cols
                    scale=1.0 / SW,
                )
                i_t[h][b2] = t

            # h=0 groups ride the a8 arrival kp-major (both banks in flight)
            psI = {b2: _ps_i(b2) for b2 in range(BT)}
            for kp in range(KP):
                for b2 in range(BT):
                    _mm_i(psI[b2], kp, 0, b2)
            for b2 in range(BT):
                _act_i(psI[b2], 0, b2)
            for h in range(1, HT):
                for b2 in range(BT):
                    ps = _ps_i(b2)
                    for kp in range(KP):
                        _mm_i(ps, kp, h, b2)
                    _act_i(ps, h, b2)
            # c_prev loads issue from the scalar ring between phase-1 ACTs
            # so their FIFO slot trails the a16/w16h0 stream.
            cp_t = []
            for h in range(HT):
                t = apool.tile([128, B], BF, tag=f"cp_{h}", name=f"cp_{h}")
                nc.scalar.dma_start(t[:], cp16[h * 128:(h + 1) * 128, :])
                cp_t.append(t)

            # --- phase 2: pure bf16 (c, f, o) + tails --------------------
            def _ps_g(setname, gi):
                return pspool.tile([128, 512], F32, tag=f"ps{setname}{gi}",
                                   name=f"ps{setname}{gi}")

            def _mm_g(ps, gi, k, h, b2):
                nc.tensor.matmul(
                    ps[:],
                    _w16(k, h, gi),
                    a16_t[k][:, b2 * 512:(b2 + 1) * 512],
                    start=(k == 0), stop=(k == KT - 1),
                )

            def _tail(h, b2, psum, chunks=1):
                """psum = [c, f, o] banks; i comes from i_t[h][b2]."""
                hs = slice(h * 128, (h + 1) * 128)
                w = 512 // chunks

                def _act(gi, fn, gname, c):
                    t = gpool.tile([128, w], F32, tag=f"g{gname}",
                                   name=f"g{gname}_{h}_{b2}_{c}")
                    # bias col: gate order (i, c, f, o) -> 1 + gi
                    nc.scalar.activation(
                        t[:], psum[gi][:, c * w:(c + 1) * w], fn,
                        bias=bias_t[:, (1 + gi) * HT + h:(1 + gi) * HT + h + 1],
                    )
                    return t

                for c in range(chunks):
                    cs = slice(b2 * 512 + c * w, b2 * 512 + (c + 1) * w)
                    gc = _act(0, _TANH, "c", c)
                    gf = _act(1, _SIG, "f", c)

                    t1 = epool.tile([128, w], F32, tag="t1", name=f"t1_{h}_{b2}_{c}")
                    nc.vector.tensor_mul(t1[:], i_t[h][b2][:, c * w:(c + 1) * w], gc[:])
                    t2 = epool.tile([128, w], F32, tag="t2", name=f"t2_{h}_{b2}_{c}")
                    nc.vector.tensor_mul(t2[:], gf[:], cp_t[h][:, cs])
                    cn = epool.tile([128, w], BF, tag="cn", name=f"cn_{h}_{b2}_{c}")
                    nc.vector.tensor_add(cn[:], t1[:], t2[:])
                    nc.gpsimd.dma_start(c_nextT[hs, cs], cn[:])

                    th = epool.tile([128, w], BF, tag="th", name=f"th_{h}_{b2}_{c}")
                    nc.scalar.activation(th[:], cn[:], _TANH)

                    go = _act(2, _SIG, "o", c)
                    hn = epool.tile([128, w], BF, tag="hn", name=f"hn_{h}_{b2}_{c}")
                    nc.vector.tensor_mul(hn[:], go[:], th[:])
                    nc.gpsimd.dma_start(h_nextT[hs, cs], hn[:])

            # h=0: both batch halves k-major (6 banks) so each freshly
            # landed (a16[k], w16[k][h0]) pair is consumed 6 matmuls at a
            # time, right as it arrives.
            setname = {0: "B", 1: "C"}
            psum0 = {b2: [_ps_g(setname[b2], gi) for gi in range(3)] for b2 in range(BT)}
            for k in range(KT):
                for gi in range(3):
                    for b2 in range(BT):
                        _mm_g(psum0[b2][gi], gi, k, 0, b2)
            for b2 in range(BT):
                _tail(0, b2, psum0[b2])

            # h>=1: sequential (h, b2) groups, bank set by batch half.
            for h in range(1, HT):
                for b2 in range(BT):
                    psum = [_ps_g(setname[b2], gi) for gi in range(3)]
                    for gi in range(3):
                        for k in range(KT):
                            _mm_g(psum[gi], gi, k, h, b2)
                    last = (h == HT - 1 and b2 == BT - 1)
                    _tail(h, b2, psum, chunks=4 if last else 1)

    nc.compile()
    return nc


_NC_CACHE = None
_LAST_IN_MAPS = None


def kernel(x, h_prev, c_prev, W_i, b_i, W_f, b_f, W_c, b_c, W_o, b_o):
    global _NC_CACHE, _LAST_IN_MAPS
    if _NC_CACHE is None:
        _NC_CACHE = _build()
    nc = _NC_CACHE

    np_e4 = mybir.dt.np(E4)
    np_bf = mybir.dt.np(BF)

    combT = np.concatenate([x, h_prev], axis=1).T          # [K, BATCH] f32
    a8_full = combT.astype(np_e4)
    a16_full = combT.astype(np_bf)

    # w8i[kp*128+p, h*256+j*128+m] = (W_i*SW)[h*128+m, kp*256+j*128+p]
    w8i = np.ascontiguousarray(
        (W_i * SW).astype(np_e4)
        .reshape(HT, 128, KP, 2, 128)      # [h, m, kp, j, p]
        .transpose(2, 4, 0, 3, 1)          # [kp, p, h, j, m]
        .reshape(KP * 128, HT * 256)
    )
    # w16[p, h*3072+k*384+gi*128+m] = W_g[h*128+m, k*128+p], gi order (c, f, o)
    w16 = np.ascontiguousarray(
        np.stack([W_c, W_f, W_o])
        .astype(np_bf)
        .reshape(3, HT, 128, KT, 128)      # [gi, h, m, k, p]
        .transpose(4, 1, 3, 0, 2)          # [p, h, k, gi, m]
        .reshape(128, HT * KT * 384)
    )
    # bias2d[m, g*HT+h] = b_g[h*128+m], gate order (i, c, f, o)
    bias2d = np.ascontiguousarray(
        np.stack([b_i, b_c, b_f, b_o])
        .reshape(4, HT, 128)
        .transpose(2, 0, 1)
        .reshape(128, 4 * HT)
    ).astype(np.float32)
    cp_full = c_prev.T.astype(np_bf)                       # [H, BATCH]

    in_maps = []
    for j in range(N_CORES):
        cols = slice(j * B, (j + 1) * B)
        a8_core = np.ascontiguousarray(
            a8_full[:, cols].reshape(KP, 2, 128, B)       # [kp, j2, p, b]
            .transpose(0, 2, 1, 3)                        # [kp, p, j2, b]
            .reshape(KP * 128, 2 * B)
        )
        in_maps.append({
            "a8": a8_core,
            "a16": np.ascontiguousarray(a16_full[:, cols]),
            "w8i": w8i,
            "w16": w16,
            "bias2d": bias2d,
            "cp16": np.ascontiguousarray(cp_full[:, cols]),
        })

    _LAST_IN_MAPS = in_maps
    try:
        res = run_bass_kernel_spmd(nc, in_maps, core_ids=list(range(N_CORES)))
    except Exception:
        # transient NRT_EXEC_UNIT_UNRECOVERABLE has been observed once on an
        # otherwise-correct NEFF; one retry is cheap insurance.
        res = run_bass_kernel_spmd(nc, in_maps, core_ids=list(range(N_CORES)))

    h_next = np.concatenate([r["h_nextT"].T for r in res.results], axis=0)
    c_next = np.concatenate([r["c_nextT"].T for r in res.results], axis=0)
    return (h_next.astype(np.float32), c_next.astype(np.float32))


# revision 9
# speedup vs baseline: 1.0808x; 1.0736x over previous
"""LSTM cell (batch 8192, input 512, hidden 512) on 8 Trainium2 NeuronCores.

Data-parallel over the batch dim: each core handles 1024 rows; weights are
replicated. Everything is computed in [hidden, batch] layout with the
contraction dim (fan_in = 1024) on SBUF partitions:

  gate.T[n, b] = sum_k W.T[k, n] * combined.T[k, b]     (matmul: lhsT.T @ rhs)

Precision plan (measured rel-err 1.55e-2 vs the 2e-2 budget): the i-gate
matmul runs in fp8-e4m3 with MatmulPerfMode.DoubleRow (K=256 per
instruction at the same ~216ns as a K=128 bf16 matmul = 2x MACs), weights
pre-scaled by 128 and descaled via the ACT scale operand. The f/c/o gates
stay bf16: their error amplification (f multiplies c_prev, c~ passes
tanh' = 1, o hits h directly) makes fp8 too lossy for them. Gates are f32;
cn/th/hn and both outputs are bf16; c_prev ships bf16.

Schedule facts this version is built around (measured on HW):
- PE dtype switches (bf16 <-> fp8) cost ~400ns of PE stall, so ALL fp8
  work runs as one up-front phase (i-gates of all 8 groups, parked in
  SBUF), then one pure-bf16 phase. 1 switch instead of 16.
- Engine-issued DMAs serialize through one global FIFO (~275 GB/s) in
  issue order, so issue order == need order: w8i/a8 pairs (phase 1)
  first, then (a16[k], w16[k][h0]) pairs k-major feeding the h=0 bf16
  ramp, then per-h weight strips + c_prev, outputs on their own ring.
- Per-core PE floor: 32 DR + 192 bf16 matmuls x 512 cols ~ 48.5us.
"""

import numpy as np

import concourse.bacc as bacc
import concourse.bass as bass
import concourse.mybir as mybir
from concourse import tile
from concourse.bass_utils import run_bass_kernel_spmd

N_CORES = 8
BATCH = 8192
B = BATCH // N_CORES  # 1024 batch rows per core
K = 1024              # fan_in = input_dim + hidden_dim
H = 512               # hidden dim
KT = K // 128         # 8 bf16 contraction tiles
KP = K // 256         # 4 fp8 DoubleRow contraction tiles
HT = H // 128         # 4 hidden chunks per gate
BT = B // 512         # 2 batch halves (PSUM free-dim limit is 512 f32)
SW = 128.0            # fp8 weight pre-scale (descaled in ACT)

E4 = mybir.dt.float8e4
BF = mybir.dt.bfloat16
F32 = mybir.dt.float32
DR = mybir.MatmulPerfMode.DoubleRow

_SIG = mybir.ActivationFunctionType.Sigmoid
_TANH = mybir.ActivationFunctionType.Tanh


def _build():
    nc = bacc.Bacc(
        "TRN2",
        target_bir_lowering=False,
        debug=False,
        num_devices=N_CORES,
    )

    # a8 rows kp*128+p, cols j*B + b  (j = which 128-half of the 256 k-block)
    a8 = nc.dram_tensor("a8", [KP * 128, 2 * B], E4, kind="ExternalInput")
    # a16 rows k*128+p, cols b
    a16 = nc.dram_tensor("a16", [K, B], BF, kind="ExternalInput")
    # w8i rows kp*128+p, cols h*256 + j*128 + m
    w8i = nc.dram_tensor("w8i", [KP * 128, HT * 256], E4, kind="ExternalInput")
    # w16 rows p, cols h*3072 + k*384 + gi*128 + m  (gi: 0=c, 1=f, 2=o)
    w16 = nc.dram_tensor("w16", [128, HT * KT * 384], BF, kind="ExternalInput")
    # bias2d col = g*HT + h, gate order (i, c, f, o)
    bias2d = nc.dram_tensor("bias2d", [128, 4 * HT], F32, kind="ExternalInput")
    cp16 = nc.dram_tensor("cp16", [H, B], BF, kind="ExternalInput")
    h_nextT = nc.dram_tensor("h_nextT", [H, B], BF, kind="ExternalOutput")
    c_nextT = nc.dram_tensor("c_nextT", [H, B], BF, kind="ExternalOutput")

    with tile.TileContext(nc) as tc:
        with (
            tc.tile_pool(name="acts", bufs=1) as apool,
            tc.tile_pool(name="wts", bufs=1) as wpool,
            tc.tile_pool(name="igates", bufs=1) as ipool,
            tc.tile_pool(name="gates", bufs=3) as gpool,
            tc.tile_pool(name="ew", bufs=3) as epool,
            tc.tile_pool(name="psum", bufs=1, space="PSUM") as pspool,
        ):
            # --- PE clock warm-up ----------------------------------------
            # The PE runs at 0.65/1.2GHz until ~3us of cumulative activity.
            # Burn that ramp on dummy matmuls over a memset tile while the
            # first input DMAs are still in flight (PE is idle anyway).
            warm_t = wpool.tile([128, 512], BF, tag="warm", name="warm")
            nc.vector.memset(warm_t[:], 0.0)
            ps_warm = pspool.tile([128, 512], F32, tag="psB0", name="ps_warm")
            for r in range(6):
                nc.tensor.matmul(
                    ps_warm[:], warm_t[:, 0:128], warm_t[:],
                    start=(r == 0), stop=(r == 5),
                )
            warm_o = wpool.tile([128, 512], F32, tag="warm_o", name="warm_o")
            nc.vector.tensor_scalar_mul(warm_o[:], ps_warm[:], 0.0)

            # --- input DMA, in need order ---------------------------------
            # scalar ring: fp8 weights (phase-1 critical path) then bias;
            # the scalar engine is otherwise idle until the first ACT.
            w8i_t = []
            for kp in range(KP):
                t = wpool.tile([128, HT, 2, 128], E4, tag=f"w8_{kp}", name=f"w8_{kp}")
                nc.scalar.dma_start(t[:], w8i[kp * 128:(kp + 1) * 128, :])
                w8i_t.append(t)
            bias_t = wpool.tile([128, 4 * HT], F32, tag="bias", name="bias")
            nc.scalar.dma_start(bias_t[:], bias2d[:])

            # sync ring: fp8 activations, then (a16[k], w16[k][h0]) pairs
            # k-major -- the h=0 bf16 phase consumes exactly in this order --
            # then the batched h>=1 weight strips.
            a8_t = []
            for kp in range(KP):
                t = apool.tile([128, 2, B], E4, tag=f"a8_{kp}", name=f"a8_{kp}")
                nc.sync.dma_start(t[:], a8[kp * 128:(kp + 1) * 128, :])
                a8_t.append(t)
            a16_t = [None] * KT
            w16h0_t = [None] * KT
            for k in range(KT):
                t = apool.tile([128, B], BF, tag=f"a16_{k}", name=f"a16_{k}")
                nc.sync.dma_start(t[:], a16[k * 128:(k + 1) * 128, :])
                a16_t[k] = t
                wt = wpool.tile([128, 3, 128], BF, tag=f"w16h0_{k}", name=f"w16h0_{k}")
                nc.sync.dma_start(wt[:], w16[:, k * 384:(k + 1) * 384])
                w16h0_t[k] = wt
            # c_prev + batched h>=1 weight strips, interleaved so each cp
            # tile's FIFO slot precedes the weights of the NEXT h (cp[h] is
            # first needed by the h-tail, well after that h's matmuls).
            w16h_t = [None] * HT
            cp_t = [None] * HT
            def _load_cp(h):
                t = apool.tile([128, B], BF, tag=f"cp_{h}", name=f"cp_{h}")
                nc.sync.dma_start(t[:], cp16[h * 128:(h + 1) * 128, :])
                cp_t[h] = t
            _load_cp(0)
            for h in range(1, HT):
                wt = wpool.tile([128, KT, 3, 128], BF, tag=f"w16h_{h}", name=f"w16h_{h}")
                nc.sync.dma_start(wt[:], w16[:, h * 3072:(h + 1) * 3072])
                w16h_t[h] = wt
                _load_cp(h)

            def _w16(k, h, gi):
                if h == 0:
                    return w16h0_t[k][:, gi, :]
                return w16h_t[h][:, k, gi, :]

            # --- phase 1: all i-gate fp8 DoubleRow matmuls ---------------
            # Results parked as f32 SBUF tiles; ~7us of PE, one dtype
            # switch when the bf16 phase starts.
            i_t = [[None] * BT for _ in range(HT)]

            def _ps_i(par):
                return pspool.tile([128, 512], F32, tag=f"psI{par}", name=f"psI{par}")

            def _mm_i(ps, kp, h, b2):
                nc.tensor.matmul(
                    ps[:],
                    w8i_t[kp][:, h, :, :],
                    a8_t[kp][:, :, b2 * 512:(b2 + 1) * 512],
                    start=(kp == 0), stop=(kp == KP - 1),
                    perf_mode=DR,
                )

            def _act_i(ps, h, b2):
                t = ipool.tile([128, 512], F32, tag=f"i_{h}_{b2}", name=f"i_{h}_{b2}")
                nc.scalar.activation(
                    t[:], ps[:], _SIG,
                    bias=bias_t[:, h:h + 1],  # gate 0 cols
                    scale=1.0 / SW,
                )
                i_t[h][b2] = t

            # h=0 groups ride the a8 arrival kp-major (both banks in flight)
            psI = {b2: _ps_i(b2) for b2 in range(BT)}
            for kp in range(KP):
                for b2 in range(BT):
                    _mm_i(psI[b2], kp, 0, b2)
            for b2 in range(BT):
                _act_i(psI[b2], 0, b2)
            for h in range(1, HT):
                for b2 in range(BT):
                    ps = _ps_i(b2)
                    for kp in range(KP):
                        _mm_i(ps, kp, h, b2)
                    _act_i(ps, h, b2)

            # --- phase 2: pure bf16 (c, f, o) + tails --------------------
            def _ps_g(setname, gi):
                return pspool.tile([128, 512], F32, tag=f"ps{setname}{gi}",
                                   name=f"ps{setname}{gi}")

            def _mm_g(ps, gi, k, h, b2):
                nc.tensor.matmul(
                    ps[:],
                    _w16(k, h, gi),
                    a16_t[k][:, b2 * 512:(b2 + 1) * 512],
                    start=(k == 0), stop=(k == KT - 1),
                )

            def _tail(h, b2, psum, split_o=False):
                """psum = [c, f, o] banks; i comes from i_t[h][b2].

                split_o: for the final group, everything through tanh(c_next)
                runs full-width (overlapping the o-gate matmuls still on the
                PE); only the post-o chain (ACT o -> hn -> DMA) is split in
                two 256-col chunks on separate DMA rings to shorten the
                drain after the last matmul.
                """
                hs = slice(h * 128, (h + 1) * 128)
                cs = slice(b2 * 512, (b2 + 1) * 512)

                def _act(gi, fn, gname, lo=0, w=512):
                    t = gpool.tile([128, w], F32, tag=f"g{gname}",
                                   name=f"g{gname}_{h}_{b2}_{lo}")
                    # bias col: gate order (i, c, f, o) -> 1 + gi
                    nc.scalar.activation(
                        t[:], psum[gi][:, lo:lo + w], fn,
                        bias=bias_t[:, (1 + gi) * HT + h:(1 + gi) * HT + h + 1],
                    )
                    return t

                gc = _act(0, _TANH, "c")
                gf = _act(1, _SIG, "f")

                t1 = epool.tile([128, 512], F32, tag="t1", name=f"t1_{h}_{b2}")
                nc.vector.tensor_mul(t1[:], i_t[h][b2][:], gc[:])
                t2 = epool.tile([128, 512], F32, tag="t2", name=f"t2_{h}_{b2}")
                nc.vector.tensor_mul(t2[:], gf[:], cp_t[h][:, cs])
                cn = epool.tile([128, 512], BF, tag="cn", name=f"cn_{h}_{b2}")
                nc.vector.tensor_add(cn[:], t1[:], t2[:])
                nc.gpsimd.dma_start(c_nextT[hs, cs], cn[:])

                th = epool.tile([128, 512], BF, tag="th", name=f"th_{h}_{b2}")
                nc.scalar.activation(th[:], cn[:], _TANH)

                if not split_o:
                    go = _act(2, _SIG, "o")
                    hn = epool.tile([128, 512], BF, tag="hn", name=f"hn_{h}_{b2}")
                    nc.vector.tensor_mul(hn[:], go[:], th[:])
                    nc.gpsimd.dma_start(h_nextT[hs, cs], hn[:])
                else:
                    for ci, eng in ((0, nc.gpsimd), (1, nc.sync)):
                        lo = ci * 256
                        go = _act(2, _SIG, "o", lo=lo, w=256)
                        hn = epool.tile([128, 256], BF, tag=f"hn{ci}",
                                        name=f"hn_{h}_{b2}_{ci}")
                        nc.vector.tensor_mul(hn[:], go[:], th[:, lo:lo + 256])
                        eng.dma_start(
                            h_nextT[hs, b2 * 512 + lo:b2 * 512 + lo + 256], hn[:])

            # h=0: both batch halves k-major (6 banks) so each freshly
            # landed (a16[k], w16[k][h0]) pair is consumed 6 matmuls at a
            # time, right as it arrives.
            setname = {0: "B", 1: "C"}
            psum0 = {b2: [_ps_g(setname[b2], gi) for gi in range(3)] for b2 in range(BT)}
            for k in range(KT):
                for gi in range(3):
                    for b2 in range(BT):
                        _mm_g(psum0[b2][gi], gi, k, 0, b2)
            for b2 in range(BT):
                _tail(0, b2, psum0[b2])

            # h>=1: sequential (h, b2) groups, bank set by batch half.
            for h in range(1, HT):
                for b2 in range(BT):
                    psum = [_ps_g(setname[b2], gi) for gi in range(3)]
                    for gi in range(3):
                        for k in range(KT):
                            _mm_g(psum[gi], gi, k, h, b2)
                    last = (h == HT - 1 and b2 == BT - 1)
                    _tail(h, b2, psum, split_o=last)

    nc.compile()
    return nc


_NC_CACHE = None
_LAST_IN_MAPS = None


def kernel(x, h_prev, c_prev, W_i, b_i, W_f, b_f, W_c, b_c, W_o, b_o):
    global _NC_CACHE, _LAST_IN_MAPS
    if _NC_CACHE is None:
        _NC_CACHE = _build()
    nc = _NC_CACHE

    np_e4 = mybir.dt.np(E4)
    np_bf = mybir.dt.np(BF)

    combT = np.concatenate([x, h_prev], axis=1).T          # [K, BATCH] f32
    a8_full = combT.astype(np_e4)
    a16_full = combT.astype(np_bf)

    # w8i[kp*128+p, h*256+j*128+m] = (W_i*SW)[h*128+m, kp*256+j*128+p]
    w8i = np.ascontiguousarray(
        (W_i * SW).astype(np_e4)
        .reshape(HT, 128, KP, 2, 128)      # [h, m, kp, j, p]
        .transpose(2, 4, 0, 3, 1)          # [kp, p, h, j, m]
        .reshape(KP * 128, HT * 256)
    )
    # w16[p, h*3072+k*384+gi*128+m] = W_g[h*128+m, k*128+p], gi order (c, f, o)
    w16 = np.ascontiguousarray(
        np.stack([W_c, W_f, W_o])
        .astype(np_bf)
        .reshape(3, HT, 128, KT, 128)      # [gi, h, m, k, p]
        .transpose(4, 1, 3, 0, 2)          # [p, h, k, gi, m]
        .reshape(128, HT * KT * 384)
    )
    # bias2d[m, g*HT+h] = b_g[h*128+m], gate order (i, c, f, o)
    bias2d = np.ascontiguousarray(
        np.stack([b_i, b_c, b_f, b_o])
        .reshape(4, HT, 128)
        .transpose(2, 0, 1)
        .reshape(128, 4 * HT)
    ).astype(np.float32)
    cp_full = c_prev.T.astype(np_bf)                       # [H, BATCH]

    in_maps = []
    for j in range(N_CORES):
        cols = slice(j * B, (j + 1) * B)
        a8_core = np.ascontiguousarray(
            a8_full[:, cols].reshape(KP, 2, 128, B)       # [kp, j2, p, b]
            .transpose(0, 2, 1, 3)                        # [kp, p, j2, b]
            .reshape(KP * 128, 2 * B)
        )
        in_maps.append({
            "a8": a8_core,
            "a16": np.ascontiguousarray(a16_full[:, cols]),
            "w8i": w8i,
            "w16": w16,
            "bias2d": bias2d,
            "cp16": np.ascontiguousarray(cp_full[:, cols]),
        })

    _LAST_IN_MAPS = in_maps
    try:
        res = run_bass_kernel_spmd(nc, in_maps, core_ids=list(range(N_CORES)))
    except Exception:
        # transient NRT_EXEC_UNIT_UNRECOVERABLE has been observed once on an
        # otherwise-correct NEFF; one retry is cheap insurance.
        res = run_bass_kernel_spmd(nc, in_maps, core_ids=list(range(N_CORES)))

    h_next = np.concatenate([r["h_nextT"].T for r in res.results], axis=0)
    c_next = np.concatenate([r["c_nextT"].T for r in res.results], axis=0)
    return (h_next.astype(np.float32), c_next.astype(np.float32))
